# revision 62
# baseline (speedup 1.0000x reference)
"""Trainium2 Bass kernel for DigitConvolutionalModel:
    out = relu(conv2d_3x3_valid(x.reshape(B,28,28))) .reshape(B,676) @ W + b

Strategy (pure data parallel over 8 cores, B=32768 -> 4096/core), v8:

Key perf facts (measured via exp_time3.py For_i-slope, ~39 us/pass vs
92 us baseline):
  * PE tile_size uniformity is critical: mixing (64,128) and (128,128)
    matmuls costs ~355 ns per transition (~30 us/pass!).  Everything
    here - conv mains, conv spills, FC strips - uses K=112 zero-padded
    weights so the stationary tile stays one shape per phase.
  * DMA floor ~23 us (6.42 MB fp16/core), conv PE floor ~22 us: this
    problem sits on the memory/compute ridge by design.

Per core, samples are processed in 8 groups of N=512 (sample order is the
identity: group g column n = sample 512*g + n). The marshaled x puts one
sample's row-group t (4 image rows x 28 cols = 112 pixels) on 112
partitions, with free dim (t, n):
    x_d[g, 28*rl + c, 512*t + n] = x[512*g + n, 28*(4t+rl) + c]
so a group loads with one contiguous DMA per ring (7168 B per partition,
split into two column-halves on the two HWDGE rings sync + scalar; Tile's
byte-range dep tracking lets early conv chunks start off the first half).

Conv: output chunk t = output rows 4t..4t+3 (t=6: rows 24..25), M =
26*il + c padded to 128. Chunk t needs input rows 4t..4t+5: rows
4t..4t+3 come from row-group t, rows 4t+4/4t+5 from the first 56
partitions of row-group t+1 (PSUM-accumulated spill matmul).  ALL
matmuls use K=112 (spillT is zero-padded rows 56..111) so the PE never
switches tile_size - mixed (64,128)/(128,128) tiles measured ~355 ns
extra per transition.  13 matmuls of N=512 per group.

Relu evacuates each conv chunk's PSUM [112, 512] into fp16 h_t tiles
(rows 104..111 stay zero since mainT/spillT cols 104..127 are zero),
alternating DVE/ACT.

FC is 4-way column-tiled with K=112 (wsbT zero-padded), one strip per
group per chunk, accumulating into psf[128, 512]; no bias matmul - the
bias rides the PSUM evacuation (DVE tensor_scalar_add with a
per-partition bias column).

Output: one contiguous [128, 512] fp32 store per batch of 4 groups
(out_d[batch, 32*gg + c, n]; rows c>=10 are junk) - small low-partition
store DMAs pay ~2 us HBM completion latency each, one wide store
doesn't.  The host slices/transposes back to [4096, 10] (free).

PSUM: 7 conv banks + 1 FC bank (all 8); 6 conv banks makes the 7th
chunk's matmul wait on a relu and costs ~1 us.  Relu alternates
DVE/ACT per chunk parity - all-DVE gates PSUM bank recycling.
"""

import sys
import numpy as np

for _p in ("/opt/trn_rl_repo", "/root/.axon_site/_ro/trn_rl_repo"):
    if _p not in sys.path:
        sys.path.insert(0, _p)

import concourse.bass as bass  # noqa: E402,F401
import concourse.tile as tile  # noqa: E402
from concourse import bacc, mybir  # noqa: E402
from concourse.bass_utils import run_bass_kernel_spmd  # noqa: E402

IMG = 28
KW = 3
OUT = 26  # IMG - KW + 1
NPIX = IMG * IMG          # 784
NOUTPIX = OUT * OUT       # 676
NCLS = 10
NCORES = 8
B_TOTAL = 32768
B_CORE = B_TOTAL // NCORES   # 4096
NG = 8                       # groups per core
N = 512                      # samples per group
NT = 7                       # row-groups (4 rows x 28 cols = 112 partitions)
CH = 104                     # features per conv chunk (4 out rows x 26)
F32 = mybir.dt.float32
F16 = mybir.dt.float16

_CACHE = {}


def _chunk_m(t):
    """Valid output rows (M) of chunk t: 104 for t<6, 52 for t=6."""
    return 52 if t == NT - 1 else 104


def _build_program(mm_dtype=F16, hwloop=0, stage=5, predma=False,
                   dma_groups=1, dma_eng="split3c", warmup=6,
                   relu_eng="mix", evac_eng="dve", hoist=0, store_mode="wide",
                   conv_variant="padk", pair_mode="full",
                   fc_batch=4, convbufs=7, fcbufs=1):
    """Build + compile the per-core Bass program (identical on all cores)."""
    nc = bacc.Bacc("TRN2", target_bir_lowering=False, debug=False,
                   num_devices=NCORES)

    x_d = nc.dram_tensor("x", (NG, 112, NT * N), mm_dtype,
                         kind="ExternalInput")
    pair = conv_variant == "pair"
    colspill = conv_variant == "colspill"
    if pair:
        pair_d = nc.dram_tensor("pairW", (128, NT * 128), mm_dtype,
                                kind="ExternalInput")
        wsb_d = nc.dram_tensor("wsbP", (128, NT * 32), mm_dtype,
                               kind="ExternalInput")
    elif colspill:
        main_d = nc.dram_tensor("mainC", (112, NT * 128), mm_dtype,
                                kind="ExternalInput")
        spill_d = nc.dram_tensor("spillC", (112, (NT - 1) * 64), mm_dtype,
                                 kind="ExternalInput")
        wsb_d = nc.dram_tensor("wsbF", (128, NT * 32), mm_dtype,
                               kind="ExternalInput")
    else:
        main_d = nc.dram_tensor("mainT", (112, NT * 128), mm_dtype,
                                kind="ExternalInput")
        spill_d = nc.dram_tensor("spillT", (112, (NT - 1) * 128), mm_dtype,
                                 kind="ExternalInput")
        wsb_d = nc.dram_tensor("wsbT", (112, NT * 32), mm_dtype,
                               kind="ExternalInput")
    bias_d = nc.dram_tensor("biascol", (128, 1), F32, kind="ExternalInput")
    out_d = nc.dram_tensor("out", (NG // fc_batch, 32 * fc_batch, N), F32,
                           kind="ExternalOutput")

    x_ap = x_d.ap()
    out_ap = out_d.ap()
    GPB = fc_batch                      # groups per FC batch

    with tile.TileContext(nc) as tc:
        with (
            tc.tile_pool(name="consts", bufs=1) as consts,
            tc.tile_pool(name="xin", bufs=max(2, NG // dma_groups)) as xin,
            tc.tile_pool(name="hbuf", bufs=2) as hbuf,
            tc.tile_pool(name="obuf", bufs=2) as obuf,
            tc.tile_pool(name="convps", bufs=convbufs, space="PSUM") as convps,
            tc.tile_pool(name="fcps", bufs=fcbufs, space="PSUM") as fcps,
        ):
            if pair:
                pairW = consts.tile([128, NT * 128], mm_dtype)
                wsbT = consts.tile([128, NT * 32], mm_dtype)
                nc.sync.dma_start(out=pairW[:, :], in_=pair_d.ap())
            elif colspill:
                mainT = consts.tile([112, NT * 128], mm_dtype)
                spillT = consts.tile([112, (NT - 1) * 64], mm_dtype)
                wsbT = consts.tile([128, NT * 32], mm_dtype)
                nc.sync.dma_start(out=mainT[:, :], in_=main_d.ap())
                nc.sync.dma_start(out=spillT[:, :], in_=spill_d.ap())
            else:
                mainT = consts.tile([112, NT * 128], mm_dtype)
                spillT = consts.tile([112, (NT - 1) * 128], mm_dtype)
                wsbT = consts.tile([112, NT * 32], mm_dtype)
                # consts split across rings so neither delays g0's x half
                # by the full 420 KB
                nc.sync.dma_start(out=mainT[:, :], in_=main_d.ap())
                nc.scalar.dma_start(out=spillT[:, :], in_=spill_d.ap())
            biascol = consts.tile([128, 1], F32)
            nc.scalar.dma_start(out=wsbT[:, :], in_=wsb_d.ap())
            nc.sync.dma_start(out=biascol[:, :], in_=bias_d.ap())

            xpre = {}
            if predma:
                for g in range(NG):
                    xp = consts.tile([112, NT * N], mm_dtype, name=f"xp{g}")
                    nc.sync.dma_start(out=xp[:, :], in_=x_ap[g])
                    xpre[g] = (xp, 0)

            import contextlib
            loop_cm = (tc.For_i(0, hwloop, 1) if hwloop
                       else contextlib.nullcontext())
            with loop_cm:
                # ---- PE warmup: un-throttle HAM while first DMA lands ----
                if warmup:
                    wu = fcps.tile([128, N], F32, tag="psf")
                    wsrc = pairW if pair else mainT
                    for _ in range(warmup):
                        nc.tensor.matmul(wu[0:128, 0:N],
                                         wsrc[0:112, 0:128],
                                         wsrc[0:112, 0:N],
                                         start=True, stop=True)
                def issue_x_dma(g):
                    if predma:
                        xts[g] = xpre[g]
                    elif g % dma_groups == 0:
                        QP = 128 if pair else 112
                        xt = xin.tile([QP, dma_groups * NT * N],
                                      mm_dtype, tag="xt")
                        if pair:
                            # rows 0-55 -> partitions 0-55, rows 56-111 ->
                            # partitions 64-119 (row-tile bases 0 / 64);
                            # each range split in halves on the two rings.
                            assert dma_groups == 1
                            H = NT * N // 2
                            for si, eng in enumerate(
                                    (nc.sync, nc.scalar)):
                                lo, hi = si * H, (si + 1) * H
                                eng.dma_start(
                                    out=xt[0:56, lo:hi],
                                    in_=x_ap[g, 0:56, lo:hi])
                                eng.dma_start(
                                    out=xt[64:120, lo:hi],
                                    in_=x_ap[g, 56:112, lo:hi])
                        elif dma_eng.startswith("split3"):
                            # thirds: two HWDGE rings + the SWDGE queue
                            bounds = {
                                "split3": [0, 1024, 2048, NT * N],
                                "split3a": [0, 1280, 2560, NT * N],
                                "split3b": [0, 1536, 3072, NT * N],
                                "split3c": [0, 896, 1792, NT * N],
                            }[dma_eng]
                            engs = (nc.sync, nc.scalar, nc.gpsimd)
                            for si, eng in enumerate(engs):
                                lo, hi = bounds[si], bounds[si + 1]
                                eng.dma_start(
                                    out=xt[:, :].rearrange(
                                        "q (g n) -> q g n",
                                        g=dma_groups)[:, :, lo:hi],
                                    in_=x_ap[g:g + dma_groups, :, lo:hi]
                                    .rearrange("g q n -> q g n"))
                        elif dma_eng == "split":
                            # halves of the free dim on both HWDGE rings
                            H = NT * N // 2
                            for si, eng in enumerate(
                                    (nc.sync, nc.scalar)):
                                eng.dma_start(
                                    out=xt[:, :].rearrange(
                                        "q (g n) -> q g n",
                                        g=dma_groups)[:, :,
                                                      si * H:
                                                      (si + 1) * H],
                                    in_=x_ap[g:g + dma_groups, :,
                                             si * H:(si + 1) * H]
                                    .rearrange("g q n -> q g n"))
                        else:
                            eng = (nc.sync if (g // dma_groups) % 2
                                   == 0 else nc.scalar)
                            eng.dma_start(
                                out=xt[:, :].rearrange(
                                    "q (g n) -> q g n", g=dma_groups),
                                in_=x_ap[g:g + dma_groups].rearrange(
                                    "g q n -> q g n"))
                        for k in range(dma_groups):
                            xts[g + k] = (xt, k * NT * N)

                xts = {}
                if hoist:
                    for g in range(NG):
                        issue_x_dma(g)
                for batch in range(NG // GPB):
                    h_all = []
                    for gg in range(GPB):
                        g = batch * GPB + gg
                        if not hoist:
                            issue_x_dma(g)
                        xtile, xoff = xts[g]
                        if stage < 2:
                            continue
                        xv = xtile[:, xoff:xoff + NT * N].rearrange(
                            "q (t n) -> q t n", n=N)

                        # ---- conv: 13 matmuls (7 main + 6 spill) ----
                        # All K=112 (spillT zero-padded) so tile_size never
                        # changes.  Issue order m0 m1 s0 m2 s1 ... so each
                        # accumulating spill lands >=2 matmuls after its
                        # main.
                        hts = {}
                        pqs = {}
                        QH = 128 if (pair or colspill) else 112

                        def relu_t(t):
                            ht = hbuf.tile([QH, N], mm_dtype,
                                           tag=f"h{gg}_{t}")
                            if relu_eng == "dve" or t % 2 == 0:
                                nc.vector.tensor_scalar_max(
                                    ht[0:QH, :], pqs[t][0:QH, 0:N], 0.0)
                            else:
                                nc.scalar.activation(
                                    ht[0:QH, :], pqs[t][0:QH, 0:N],
                                    mybir.ActivationFunctionType.Relu)
                            hts[t] = ht

                        if pair:
                            pm = pair_mode
                            # 2x2 (row, col)-tiled K=56/M=64 matmuls: bank
                            # k holds out-row-pair 2k (partitions 0-63)
                            # and 2k+1 (partitions 64-127); each pair =
                            # two accumulating row-tiles (bases 0 / 64).
                            # All 4 MMs of a bank can overlap in the PE
                            # (disjoint 64x64 array quadrants).
                            for k in range(NT):
                                pq = convps.tile([128, N], F32, tag="pq")
                                pqs[k] = pq
                                C = 128 * k
                                # pair 2k: input rows 4k..4k+3 (block k)
                                nc.tensor.matmul(
                                    pq[0:64, 0:N],
                                    pairW[0:56, C:C + 64],
                                    xv[0:56, k, :],
                                    start=True, stop=(pm == "a1"),
                                    tile_position=(0, 0))
                                if pm == "a1":
                                    if stage >= 3:
                                        relu_t(k)
                                    continue
                                nc.tensor.matmul(
                                    pq[0:64, 0:N],
                                    pairW[64:120, C:C + 64],
                                    xv[64:120, k, :],
                                    start=False, stop=True,
                                    tile_position=(64, 0))
                                if pm == "aonly":
                                    if stage >= 3:
                                        relu_t(k)
                                    continue
                                if k < NT - 1:
                                    # pair 2k+1: rows 4k+2..4k+5
                                    nc.tensor.matmul(
                                        pq[64:128, 0:N],
                                        pairW[64:120, C + 64:C + 128],
                                        xv[64:120, k, :],
                                        start=True, stop=False,
                                        tile_position=(64, 64))
                                    nc.tensor.matmul(
                                        pq[64:128, 0:N],
                                        pairW[0:56, C + 64:C + 128],
                                        xv[0:56, k + 1, :],
                                        start=False, stop=True,
                                        tile_position=(0, 64))
                                else:
                                    # zero-weight filler writes exact 0s
                                    # so relu/FC never see stale PSUM
                                    nc.tensor.matmul(
                                        pq[64:128, 0:N],
                                        pairW[0:56, C + 64:C + 128],
                                        xv[0:56, k, :],
                                        start=True, stop=True,
                                        tile_position=(0, 64))
                                if stage >= 3:
                                    relu_t(k)
                        elif colspill:
                            # mains (128,128); spills M=52 col-tiles in
                            # concurrent pairs: even chunk's spill window
                            # at cols 0-51 (tile_position (0,0)), odd at
                            # 64-115 ((0,64)) - same proven pattern as
                            # the FC strips, different banks per spill.
                            order = [("m", 0), ("m", 1), ("m", 2),
                                     ("s", 0), ("s", 1), ("m", 3),
                                     ("m", 4), ("s", 2), ("s", 3),
                                     ("m", 5), ("m", 6), ("s", 4),
                                     ("s", 5)]
                            for kind, t in order:
                                if kind == "m":
                                    pq = convps.tile([128, N], F32,
                                                     tag="pq")
                                    pqs[t] = pq
                                    nc.tensor.matmul(
                                        pq[0:128, 0:N],
                                        mainT[0:112,
                                              128 * t:128 * t + 128],
                                        xv[:, t, :],
                                        start=True,
                                        stop=(t == NT - 1),
                                    )
                                    if stage >= 3 and t == NT - 1:
                                        relu_t(t)
                                else:
                                    side = 0 if t % 2 == 0 else 64
                                    nc.tensor.matmul(
                                        pqs[t][side:side + 52, 0:N],
                                        spillT[0:112,
                                               64 * t:64 * t + 52],
                                        xv[0:112, t + 1, :],
                                        start=False, stop=True,
                                        tile_position=(0, side),
                                    )
                                    if stage >= 3:
                                        relu_t(t)
                        else:
                            order = [("m", 0)]
                            for t in range(1, NT):
                                order += [("m", t), ("s", t - 1)]
                            for kind, t in order:
                                if kind == "m":
                                    pq = convps.tile([128, N], F32,
                                                     tag="pq")
                                    pqs[t] = pq
                                    nc.tensor.matmul(
                                        pq[0:128, 0:N],
                                        mainT[0:112,
                                              128 * t:128 * t + 128],
                                        xv[:, t, :],
                                        start=True,
                                        stop=(t == NT - 1),
                                    )
                                    if stage >= 3 and t == NT - 1:
                                        relu_t(t)
                                else:
                                    nc.tensor.matmul(
                                        pqs[t][0:128, 0:N],
                                        spillT[0:112,
                                               128 * t:128 * t + 128],
                                        xv[0:112, t + 1, :],
                                        start=False, stop=True,
                                    )
                                    if stage >= 3:
                                        relu_t(t)
                        h_all.append([hts[t] for t in sorted(hts)]
                                     if stage >= 3 else [])

                    if stage < 4:
                        continue
                    # ---- FC: col-tiled strip matmuls ----
                    KFC = 128 if (pair or colspill) else 112
                    psf = fcps.tile([32 * GPB, N], F32, tag="psf")
                    for t in range(NT):
                        for gg in range(GPB):
                            nc.tensor.matmul(
                                psf[32 * gg:32 * gg + 32, 0:N],
                                wsbT[0:KFC, 32 * t:32 * t + 32],
                                h_all[gg][t][0:KFC, :],
                                start=(t == 0), stop=(t == NT - 1),
                                tile_position=(0, 32 * gg),
                            )
                    if stage < 5:
                        continue
                    # ---- tail: evac (+bias), class-major store ----
                    osb = obuf.tile([32 * GPB, N], F32, tag="osb")
                    if evac_eng == "dve":
                        nc.vector.tensor_scalar_add(
                            osb[:, :], psf[0:32 * GPB, 0:N],
                            biascol[0:32 * GPB, 0:1])
                    else:
                        nc.scalar.activation(
                            osb[:, :], psf[0:32 * GPB, 0:N],
                            mybir.ActivationFunctionType.Identity,
                            bias=biascol[0:32 * GPB, 0:1])
                    if store_mode == "wide":
                        # ONE contiguous [128, 512] store per batch (junk
                        # rows included; host slices rows 32*gg+c).
                        nc.sync.dma_start(out=out_ap[batch],
                                          in_=osb[0:32 * GPB, :])
                    else:
                        for gg in range(GPB):
                            eng = (nc.sync if (store_mode != "narrow2"
                                               or gg % 2 == 0)
                                   else nc.scalar)
                            eng.dma_start(
                                out=out_ap[batch,
                                           32 * gg:32 * gg + NCLS, :],
                                in_=osb[32 * gg:32 * gg + NCLS, :])

    nc.compile()
    return nc


def _host_constants(conv_w, W, b):
    """Conv chunk lhsTs (main + spill), FC chunk lhsTs, bias column."""
    mainT = np.zeros((112, NT * 128), np.float32)
    spillT = np.zeros((112, (NT - 1) * 128), np.float32)
    for t in range(NT):
        nil = 4 if t < NT - 1 else 2
        for il in range(nil):
            for c in range(OUT):
                mcol = 26 * il + c
                for rl in range(4):
                    di = rl - il
                    if not (0 <= di < KW):
                        continue
                    for cq in range(c, c + KW):
                        mainT[28 * rl + cq, 128 * t + mcol] = \
                            conv_w[di, cq - c]
                if t < NT - 1:
                    for rl2 in range(2):
                        di = 4 + rl2 - il
                        if not (0 <= di < KW):
                            continue
                        for cq in range(c, c + KW):
                            spillT[28 * rl2 + cq, 128 * t + mcol] = \
                                conv_w[di, cq - c]
    wsbT = np.zeros((112, NT * 32), np.float32)
    for t in range(NT):
        m = _chunk_m(t)
        wsbT[0:m, 32 * t:32 * t + NCLS] = W[CH * t:CH * t + m, :]
    biascol = np.zeros((128, 1), np.float32)
    for gg in range(4):
        biascol[32 * gg:32 * gg + NCLS, 0] = b
    return mainT, spillT, wsbT, biascol


def _host_pair_constants(conv_w, W):
    """2x2-tiled conv lhsT + matching FC lhsT.

    Bank k: out-row-pair 2k (rows 4k,4k+1) on PSUM partitions 0-63 and
    pair 2k+1 (rows 4k+2,4k+3) on partitions 64-127.  Input rows
    4k,4k+1 live at x partitions 0-55 of block k; rows 4k+2,4k+3 at
    partitions 64-119.  pairW[partition, 128k + 64*half + 26*j + c].
    """
    pairW = np.zeros((128, NT * 128), np.float32)
    wsbP = np.zeros((128, NT * 32), np.float32)
    for k in range(NT):
        for half in range(2):           # 0: pair 2k, 1: pair 2k+1
            p = 2 * k + half
            if p > 12:
                continue
            base_row = 2 * p            # first out row of the pair
            for j in range(2):          # out row base_row + j
                orow = base_row + j
                for c in range(OUT):
                    col = 128 * k + 64 * half + 26 * j + c
                    for di in range(KW):
                        r = orow + di   # input image row
                        rl = r - 4 * k  # row within block k (0..5)
                        for dj in range(KW):
                            cq = c + dj
                            if rl < 2:
                                part = 28 * rl + cq
                            elif rl < 4:
                                part = 64 + 28 * (rl - 2) + cq
                            else:       # rows 4k+4,4k+5 = block k+1 rows 0,1
                                part = 28 * (rl - 4) + cq
                            pairW[part, col] = conv_w[di, dj]
                    feat = OUT * orow + c   # conv feature index
                    wsbP[64 * half + 26 * j + c, 32 * k:32 * k + NCLS] = \
                        W[feat, :]
    return pairW, wsbP


def _marshal_x(x):
    """[B, 784] fp32 -> per-core [NG, 112, 7*512] fp16.

    x_d[core, g, 28*rl + c, 512*t + n] = x[4096*core + 512*g + n,
                                           28*(4t+rl) + c].
    """
    xs = x.reshape(NCORES, NG, N, NT, 4, IMG)    # core g n t rl c
    xs = xs.transpose(0, 1, 4, 5, 3, 2)          # core g rl c t n
    xs = np.ascontiguousarray(xs, dtype=np.float16)
    return xs.reshape(NCORES, NG, 112, NT * N)


def _unmarshal_out(res):
    """Per-core out [2, 128, 512] = (batch, 32*gg + c, n) -> [B, 10].

    sample = 4096*core + 2048*batch + 512*gg + n; rows c>=10 are junk.
    """
    outs = []
    for r in res:
        o = r.reshape(2, 4, 32, N)[:, :, 0:NCLS].transpose(0, 1, 3, 2)
        outs.append(np.ascontiguousarray(o).reshape(B_CORE, NCLS))
    return np.concatenate(outs, axis=0)


def _col_of(t, il, c):
    """Column of output (il, c) in chunk t's permuted layout.

    The spill-affected pair (il 2,3) sits at a 64-aligned slab: cols
    0-51 for even t, 64-115 for odd t; the other pair at the other slab.
    """
    if il >= 2:
        return (0 if t % 2 == 0 else 64) + 26 * (il - 2) + c
    return (64 if t % 2 == 0 else 0) + 26 * il + c


def _host_colspill_constants(conv_w, W):
    """Conv lhsTs with parity-permuted M layout + matching FC lhsT."""
    mainC = np.zeros((112, NT * 128), np.float32)
    spillC = np.zeros((112, (NT - 1) * 64), np.float32)
    wsbF = np.zeros((128, NT * 32), np.float32)
    for t in range(NT):
        nil = 4 if t < NT - 1 else 2
        for il in range(nil):
            for c in range(OUT):
                col = _col_of(t, il, c)
                for rl in range(4):
                    di = rl - il
                    if 0 <= di < KW:
                        for dj in range(KW):
                            mainC[28 * rl + c + dj, 128 * t + col] = \
                                conv_w[di, dj]
                if t < NT - 1 and il >= 2:
                    scol = 64 * t + 26 * (il - 2) + c
                    for rl2 in range(2):
                        di = 4 + rl2 - il
                        if 0 <= di < KW:
                            for dj in range(KW):
                                spillC[28 * rl2 + c + dj, scol] = \
                                    conv_w[di, dj]
                wsbF[col, 32 * t:32 * t + NCLS] = W[OUT * (4 * t + il) + c]
    return mainC, spillC, wsbF


def _make_in_maps(x, conv_w, W, b):
    """Marshaled per-core input maps (superset of any variant's inputs)."""
    mainT, spillT, wsbT, biascol = _host_constants(conv_w, W, b)
    pairW, wsbP = _host_pair_constants(conv_w, W)
    mainC, spillC, wsbF = _host_colspill_constants(conv_w, W)
    xm = _marshal_x(np.ascontiguousarray(np.asarray(x, dtype=np.float32)))
    in_maps = []
    for i in range(NCORES):
        in_maps.append({
            "x": xm[i],
            "mainT": mainT.astype(np.float16),
            "spillT": spillT.astype(np.float16),
            "wsbT": wsbT.astype(np.float16),
            "pairW": pairW.astype(np.float16),
            "wsbP": wsbP.astype(np.float16),
            "mainC": mainC.astype(np.float16),
            "spillC": spillC.astype(np.float16),
            "wsbF": wsbF.astype(np.float16),
            "biascol": biascol,
        })
    return in_maps


def _run(x, conv_w, W, b, trace=False, mm_dtype=F16):
    x = np.ascontiguousarray(np.asarray(x, dtype=np.float32))
    conv_w = np.asarray(conv_w, dtype=np.float32)
    W = np.asarray(W, dtype=np.float32)
    b = np.asarray(b, dtype=np.float32)
    assert x.shape == (B_TOTAL, NPIX), x.shape

    key = ("prog", str(mm_dtype))
    if key not in _CACHE:
        _CACHE[key] = _build_program(mm_dtype)
    nc = _CACHE[key]

    in_maps = _make_in_maps(x, conv_w, W, b)
    res = run_bass_kernel_spmd(nc, in_maps, core_ids=list(range(NCORES)),
                               trace=trace)
    out = _unmarshal_out([res.results[i]["out"] for i in range(NCORES)])
    return out, res


def kernel(x, conv_w, W, b):
    out, _ = _run(x, conv_w, W, b, trace=False)
    return out


# revision 68
# speedup vs baseline: 1.0263x; 1.0263x over previous
"""Trainium2 Bass kernel for DigitConvolutionalModel:
    out = relu(conv2d_3x3_valid(x.reshape(B,28,28))) .reshape(B,676) @ W + b

Strategy (pure data parallel over 8 cores, B=32768 -> 4096/core), v8:

Key perf facts (measured via exp_time3.py For_i-slope, ~39 us/pass vs
92 us baseline):
  * PE tile_size uniformity is critical: mixing (64,128) and (128,128)
    matmuls costs ~355 ns per transition (~30 us/pass!).  Everything
    here - conv mains, conv spills, FC strips - uses K=112 zero-padded
    weights so the stationary tile stays one shape per phase.
  * DMA floor ~23 us (6.42 MB fp16/core), conv PE floor ~22 us: this
    problem sits on the memory/compute ridge by design.

Per core, samples are processed in 8 groups of N=512 (sample order is the
identity: group g column n = sample 512*g + n). The marshaled x puts one
sample's row-group t (4 image rows x 28 cols = 112 pixels) on 112
partitions, with free dim (t, n):
    x_d[g, 28*rl + c, 512*t + n] = x[512*g + n, 28*(4t+rl) + c]
so a group loads with one contiguous DMA per ring (7168 B per partition,
split into two column-halves on the two HWDGE rings sync + scalar; Tile's
byte-range dep tracking lets early conv chunks start off the first half).

Conv: output chunk t = output rows 4t..4t+3 (t=6: rows 24..25), M =
26*il + c padded to 128. Chunk t needs input rows 4t..4t+5: rows
4t..4t+3 come from row-group t, rows 4t+4/4t+5 from the first 56
partitions of row-group t+1 (PSUM-accumulated spill matmul).  ALL
matmuls use K=112 (spillT is zero-padded rows 56..111) so the PE never
switches tile_size - mixed (64,128)/(128,128) tiles measured ~355 ns
extra per transition.  13 matmuls of N=512 per group.

Relu evacuates each conv chunk's PSUM [112, 512] into fp16 h_t tiles
(rows 104..111 stay zero since mainT/spillT cols 104..127 are zero),
alternating DVE/ACT.

FC is 4-way column-tiled with K=112 (wsbT zero-padded), one strip per
group per chunk, accumulating into psf[128, 512]; no bias matmul - the
bias rides the PSUM evacuation (DVE tensor_scalar_add with a
per-partition bias column).

Output: one contiguous [128, 512] fp32 store per batch of 4 groups
(out_d[batch, 32*gg + c, n]; rows c>=10 are junk) - small low-partition
store DMAs pay ~2 us HBM completion latency each, one wide store
doesn't.  The host slices/transposes back to [4096, 10] (free).

PSUM: 7 conv banks + 1 FC bank (all 8); 6 conv banks makes the 7th
chunk's matmul wait on a relu and costs ~1 us.  Relu alternates
DVE/ACT per chunk parity - all-DVE gates PSUM bank recycling.
"""

import sys
import numpy as np

for _p in ("/opt/trn_rl_repo", "/root/.axon_site/_ro/trn_rl_repo"):
    if _p not in sys.path:
        sys.path.insert(0, _p)

import concourse.bass as bass  # noqa: E402,F401
import concourse.tile as tile  # noqa: E402
from concourse import bacc, mybir  # noqa: E402
from concourse.bass_utils import run_bass_kernel_spmd  # noqa: E402

IMG = 28
KW = 3
OUT = 26  # IMG - KW + 1
NPIX = IMG * IMG          # 784
NOUTPIX = OUT * OUT       # 676
NCLS = 10
NCORES = 8
B_TOTAL = 32768
B_CORE = B_TOTAL // NCORES   # 4096
NG = 8                       # groups per core
N = 512                      # samples per group
NT = 7                       # row-groups (4 rows x 28 cols = 112 partitions)
CH = 104                     # features per conv chunk (4 out rows x 26)
F32 = mybir.dt.float32
F16 = mybir.dt.float16

_CACHE = {}


def _chunk_m(t):
    """Valid output rows (M) of chunk t: 104 for t<6, 52 for t=6."""
    return 52 if t == NT - 1 else 104


def _build_program(mm_dtype=F16, hwloop=0, stage=5, predma=False,
                   dma_groups=1, dma_eng="split3c", warmup=6, tail_light=0,
                   relu_eng="mix", evac_eng="dve", hoist=0,
                   store_mode="deferred",
                   conv_variant="padk", pair_mode="full",
                   fc_batch=4, convbufs=7, fcbufs=1):
    """Build + compile the per-core Bass program (identical on all cores)."""
    nc = bacc.Bacc("TRN2", target_bir_lowering=False, debug=False,
                   num_devices=NCORES)

    x_d = nc.dram_tensor("x", (NG, 112, NT * N), mm_dtype,
                         kind="ExternalInput")
    pair = conv_variant == "pair"
    colspill = conv_variant == "colspill"
    if pair:
        pair_d = nc.dram_tensor("pairW", (128, NT * 128), mm_dtype,
                                kind="ExternalInput")
        wsb_d = nc.dram_tensor("wsbP", (128, NT * 32), mm_dtype,
                               kind="ExternalInput")
    elif colspill:
        main_d = nc.dram_tensor("mainC", (112, NT * 128), mm_dtype,
                                kind="ExternalInput")
        spill_d = nc.dram_tensor("spillC", (112, (NT - 1) * 64), mm_dtype,
                                 kind="ExternalInput")
        wsb_d = nc.dram_tensor("wsbF", (128, NT * 32), mm_dtype,
                               kind="ExternalInput")
    else:
        main_d = nc.dram_tensor("mainT", (112, NT * 128), mm_dtype,
                                kind="ExternalInput")
        spill_d = nc.dram_tensor("spillT", (112, (NT - 1) * 128), mm_dtype,
                                 kind="ExternalInput")
        wsb_d = nc.dram_tensor("wsbT", (112, NT * 32), mm_dtype,
                               kind="ExternalInput")
    bias_d = nc.dram_tensor("biascol", (128, 1), F32, kind="ExternalInput")
    out_d = nc.dram_tensor("out", (NG // fc_batch, 32 * fc_batch, N), F32,
                           kind="ExternalOutput")

    x_ap = x_d.ap()
    out_ap = out_d.ap()
    GPB = fc_batch                      # groups per FC batch

    with tile.TileContext(nc) as tc:
        with (
            tc.tile_pool(name="consts", bufs=1) as consts,
            tc.tile_pool(name="xin", bufs=max(2, NG // dma_groups)) as xin,
            tc.tile_pool(name="hbuf", bufs=2) as hbuf,
            tc.tile_pool(name="obuf", bufs=2) as obuf,
            tc.tile_pool(name="convps", bufs=convbufs, space="PSUM") as convps,
            tc.tile_pool(name="fcps", bufs=fcbufs, space="PSUM") as fcps,
        ):
            if pair:
                pairW = consts.tile([128, NT * 128], mm_dtype)
                wsbT = consts.tile([128, NT * 32], mm_dtype)
                nc.sync.dma_start(out=pairW[:, :], in_=pair_d.ap())
            elif colspill:
                mainT = consts.tile([112, NT * 128], mm_dtype)
                spillT = consts.tile([112, (NT - 1) * 64], mm_dtype)
                wsbT = consts.tile([128, NT * 32], mm_dtype)
                nc.sync.dma_start(out=mainT[:, :], in_=main_d.ap())
                nc.sync.dma_start(out=spillT[:, :], in_=spill_d.ap())
            else:
                mainT = consts.tile([112, NT * 128], mm_dtype)
                spillT = consts.tile([112, (NT - 1) * 128], mm_dtype)
                wsbT = consts.tile([112, NT * 32], mm_dtype)
                # consts split across rings so neither delays g0's x half
                # by the full 420 KB
                nc.sync.dma_start(out=mainT[:, :], in_=main_d.ap())
                nc.scalar.dma_start(out=spillT[:, :], in_=spill_d.ap())
            biascol = consts.tile([128, 1], F32)
            nc.scalar.dma_start(out=wsbT[:, :], in_=wsb_d.ap())
            nc.sync.dma_start(out=biascol[:, :], in_=bias_d.ap())

            xpre = {}
            if predma:
                for g in range(NG):
                    xp = consts.tile([112, NT * N], mm_dtype, name=f"xp{g}")
                    nc.sync.dma_start(out=xp[:, :], in_=x_ap[g])
                    xpre[g] = (xp, 0)

            import contextlib
            loop_cm = (tc.For_i(0, hwloop, 1) if hwloop
                       else contextlib.nullcontext())
            with loop_cm:
                # ---- PE warmup: un-throttle HAM while first DMA lands ----
                if warmup:
                    wu = fcps.tile([128, N], F32, tag="psf")
                    wsrc = pairW if pair else mainT
                    for _ in range(warmup):
                        nc.tensor.matmul(wu[0:128, 0:N],
                                         wsrc[0:112, 0:128],
                                         wsrc[0:112, 0:N],
                                         start=True, stop=True)
                def issue_x_dma(g):
                    if predma:
                        xts[g] = xpre[g]
                    elif g % dma_groups == 0:
                        QP = 128 if pair else 112
                        xt = xin.tile([QP, dma_groups * NT * N],
                                      mm_dtype, tag="xt")
                        if pair:
                            # rows 0-55 -> partitions 0-55, rows 56-111 ->
                            # partitions 64-119 (row-tile bases 0 / 64);
                            # each range split in halves on the two rings.
                            assert dma_groups == 1
                            H = NT * N // 2
                            for si, eng in enumerate(
                                    (nc.sync, nc.scalar)):
                                lo, hi = si * H, (si + 1) * H
                                eng.dma_start(
                                    out=xt[0:56, lo:hi],
                                    in_=x_ap[g, 0:56, lo:hi])
                                eng.dma_start(
                                    out=xt[64:120, lo:hi],
                                    in_=x_ap[g, 56:112, lo:hi])
                        elif dma_eng.startswith("split3"):
                            # thirds: two HWDGE rings + the SWDGE queue
                            bounds = {
                                "split3": [0, 1024, 2048, NT * N],
                                "split3a": [0, 1280, 2560, NT * N],
                                "split3b": [0, 1536, 3072, NT * N],
                                "split3c": [0, 896, 1792, NT * N],
                                "split3d": [0, 768, 1536, NT * N],
                            }[dma_eng]
                            if tail_light and g == NG - 1:
                                # last group: small SWDGE piece so the
                                # final-arriving DMA rides the (by now
                                # drained) HWDGE rings instead
                                bounds = [0, 1536, 3072, NT * N]
                            engs = (nc.sync, nc.scalar, nc.gpsimd)
                            for si, eng in enumerate(engs):
                                lo, hi = bounds[si], bounds[si + 1]
                                eng.dma_start(
                                    out=xt[:, :].rearrange(
                                        "q (g n) -> q g n",
                                        g=dma_groups)[:, :, lo:hi],
                                    in_=x_ap[g:g + dma_groups, :, lo:hi]
                                    .rearrange("g q n -> q g n"))
                        elif dma_eng == "split":
                            # halves of the free dim on both HWDGE rings
                            H = NT * N // 2
                            for si, eng in enumerate(
                                    (nc.sync, nc.scalar)):
                                eng.dma_start(
                                    out=xt[:, :].rearrange(
                                        "q (g n) -> q g n",
                                        g=dma_groups)[:, :,
                                                      si * H:
                                                      (si + 1) * H],
                                    in_=x_ap[g:g + dma_groups, :,
                                             si * H:(si + 1) * H]
                                    .rearrange("g q n -> q g n"))
                        else:
                            eng = (nc.sync if (g // dma_groups) % 2
                                   == 0 else nc.scalar)
                            eng.dma_start(
                                out=xt[:, :].rearrange(
                                    "q (g n) -> q g n", g=dma_groups),
                                in_=x_ap[g:g + dma_groups].rearrange(
                                    "g q n -> q g n"))
                        for k in range(dma_groups):
                            xts[g + k] = (xt, k * NT * N)

                xts = {}
                deferred = []
                if hoist:
                    for g in range(NG):
                        issue_x_dma(g)
                for batch in range(NG // GPB):
                    h_all = []
                    for gg in range(GPB):
                        g = batch * GPB + gg
                        if not hoist:
                            issue_x_dma(g)
                        xtile, xoff = xts[g]
                        if stage < 2:
                            continue
                        xv = xtile[:, xoff:xoff + NT * N].rearrange(
                            "q (t n) -> q t n", n=N)

                        # ---- conv: 13 matmuls (7 main + 6 spill) ----
                        # All K=112 (spillT zero-padded) so tile_size never
                        # changes.  Issue order m0 m1 s0 m2 s1 ... so each
                        # accumulating spill lands >=2 matmuls after its
                        # main.
                        hts = {}
                        pqs = {}
                        QH = 128 if (pair or colspill) else 112

                        def relu_t(t):
                            ht = hbuf.tile([QH, N], mm_dtype,
                                           tag=f"h{gg}_{t}")
                            if relu_eng == "dve" or t % 2 == 0:
                                nc.vector.tensor_scalar_max(
                                    ht[0:QH, :], pqs[t][0:QH, 0:N], 0.0)
                            else:
                                nc.scalar.activation(
                                    ht[0:QH, :], pqs[t][0:QH, 0:N],
                                    mybir.ActivationFunctionType.Relu)
                            hts[t] = ht

                        if pair:
                            pm = pair_mode
                            # 2x2 (row, col)-tiled K=56/M=64 matmuls: bank
                            # k holds out-row-pair 2k (partitions 0-63)
                            # and 2k+1 (partitions 64-127); each pair =
                            # two accumulating row-tiles (bases 0 / 64).
                            # All 4 MMs of a bank can overlap in the PE
                            # (disjoint 64x64 array quadrants).
                            for k in range(NT):
                                pq = convps.tile([128, N], F32, tag="pq")
                                pqs[k] = pq
                                C = 128 * k
                                # pair 2k: input rows 4k..4k+3 (block k)
                                nc.tensor.matmul(
                                    pq[0:64, 0:N],
                                    pairW[0:56, C:C + 64],
                                    xv[0:56, k, :],
                                    start=True, stop=(pm == "a1"),
                                    tile_position=(0, 0))
                                if pm == "a1":
                                    if stage >= 3:
                                        relu_t(k)
                                    continue
                                nc.tensor.matmul(
                                    pq[0:64, 0:N],
                                    pairW[64:120, C:C + 64],
                                    xv[64:120, k, :],
                                    start=False, stop=True,
                                    tile_position=(64, 0))
                                if pm == "aonly":
                                    if stage >= 3:
                                        relu_t(k)
                                    continue
                                if k < NT - 1:
                                    # pair 2k+1: rows 4k+2..4k+5
                                    nc.tensor.matmul(
                                        pq[64:128, 0:N],
                                        pairW[64:120, C + 64:C + 128],
                                        xv[64:120, k, :],
                                        start=True, stop=False,
                                        tile_position=(64, 64))
                                    nc.tensor.matmul(
                                        pq[64:128, 0:N],
                                        pairW[0:56, C + 64:C + 128],
                                        xv[0:56, k + 1, :],
                                        start=False, stop=True,
                                        tile_position=(0, 64))
                                else:
                                    # zero-weight filler writes exact 0s
                                    # so relu/FC never see stale PSUM
                                    nc.tensor.matmul(
                                        pq[64:128, 0:N],
                                        pairW[0:56, C + 64:C + 128],
                                        xv[0:56, k, :],
                                        start=True, stop=True,
                                        tile_position=(0, 64))
                                if stage >= 3:
                                    relu_t(k)
                        elif colspill:
                            # mains (128,128); spills M=52 col-tiles in
                            # concurrent pairs: even chunk's spill window
                            # at cols 0-51 (tile_position (0,0)), odd at
                            # 64-115 ((0,64)) - same proven pattern as
                            # the FC strips, different banks per spill.
                            order = [("m", 0), ("m", 1), ("m", 2),
                                     ("s", 0), ("s", 1), ("m", 3),
                                     ("m", 4), ("s", 2), ("s", 3),
                                     ("m", 5), ("m", 6), ("s", 4),
                                     ("s", 5)]
                            for kind, t in order:
                                if kind == "m":
                                    pq = convps.tile([128, N], F32,
                                                     tag="pq")
                                    pqs[t] = pq
                                    nc.tensor.matmul(
                                        pq[0:128, 0:N],
                                        mainT[0:112,
                                              128 * t:128 * t + 128],
                                        xv[:, t, :],
                                        start=True,
                                        stop=(t == NT - 1),
                                    )
                                    if stage >= 3 and t == NT - 1:
                                        relu_t(t)
                                else:
                                    side = 0 if t % 2 == 0 else 64
                                    nc.tensor.matmul(
                                        pqs[t][side:side + 52, 0:N],
                                        spillT[0:112,
                                               64 * t:64 * t + 52],
                                        xv[0:112, t + 1, :],
                                        start=False, stop=True,
                                        tile_position=(0, side),
                                    )
                                    if stage >= 3:
                                        relu_t(t)
                        else:
                            order = [("m", 0)]
                            for t in range(1, NT):
                                order += [("m", t), ("s", t - 1)]
                            for kind, t in order:
                                if kind == "m":
                                    pq = convps.tile([128, N], F32,
                                                     tag="pq")
                                    pqs[t] = pq
                                    nc.tensor.matmul(
                                        pq[0:128, 0:N],
                                        mainT[0:112,
                                              128 * t:128 * t + 128],
                                        xv[:, t, :],
                                        start=True,
                                        stop=(t == NT - 1),
                                    )
                                    if stage >= 3 and t == NT - 1:
                                        relu_t(t)
                                else:
                                    nc.tensor.matmul(
                                        pqs[t][0:128, 0:N],
                                        spillT[0:112,
                                               128 * t:128 * t + 128],
                                        xv[0:112, t + 1, :],
                                        start=False, stop=True,
                                    )
                                    if stage >= 3:
                                        relu_t(t)
                        h_all.append([hts[t] for t in sorted(hts)]
                                     if stage >= 3 else [])

                    if stage < 4:
                        continue
                    # ---- FC: col-tiled strip matmuls ----
                    KFC = 128 if (pair or colspill) else 112
                    psf = fcps.tile([32 * GPB, N], F32, tag="psf")
                    for t in range(NT):
                        for gg in range(GPB):
                            nc.tensor.matmul(
                                psf[32 * gg:32 * gg + 32, 0:N],
                                wsbT[0:KFC, 32 * t:32 * t + 32],
                                h_all[gg][t][0:KFC, :],
                                start=(t == 0), stop=(t == NT - 1),
                                tile_position=(0, 32 * gg),
                            )
                    if stage < 5:
                        continue
                    # ---- tail: evac (+bias), class-major store ----
                    osb = obuf.tile([32 * GPB, N], F32, tag="osb")
                    if evac_eng == "dve":
                        nc.vector.tensor_scalar_add(
                            osb[:, :], psf[0:32 * GPB, 0:N],
                            biascol[0:32 * GPB, 0:1])
                    else:
                        nc.scalar.activation(
                            osb[:, :], psf[0:32 * GPB, 0:N],
                            mybir.ActivationFunctionType.Identity,
                            bias=biascol[0:32 * GPB, 0:1])
                    if store_mode == "deferred":
                        # Issue all stores AFTER the batch loop: a store's
                        # osb data-wait otherwise head-of-line blocks the
                        # SP sequencer, delaying the NEXT batch's sync-ring
                        # x pieces by ~10 us.  Sems still enforce the data
                        # dep; only the ring position changes.
                        deferred.append((batch, osb))
                    elif store_mode == "wide":
                        # ONE contiguous [128, 512] store per batch (junk
                        # rows included; host slices rows 32*gg+c).
                        nc.sync.dma_start(out=out_ap[batch],
                                          in_=osb[0:32 * GPB, :])
                    else:
                        for gg in range(GPB):
                            eng = (nc.sync if (store_mode != "narrow2"
                                               or gg % 2 == 0)
                                   else nc.scalar)
                            eng.dma_start(
                                out=out_ap[batch,
                                           32 * gg:32 * gg + NCLS, :],
                                in_=osb[32 * gg:32 * gg + NCLS, :])
                for batch, osb in deferred:
                    nc.sync.dma_start(out=out_ap[batch],
                                      in_=osb[0:32 * GPB, :])

    nc.compile()
    return nc


def _host_constants(conv_w, W, b):
    """Conv chunk lhsTs (main + spill), FC chunk lhsTs, bias column."""
    mainT = np.zeros((112, NT * 128), np.float32)
    spillT = np.zeros((112, (NT - 1) * 128), np.float32)
    for t in range(NT):
        nil = 4 if t < NT - 1 else 2
        for il in range(nil):
            for c in range(OUT):
                mcol = 26 * il + c
                for rl in range(4):
                    di = rl - il
                    if not (0 <= di < KW):
                        continue
                    for cq in range(c, c + KW):
                        mainT[28 * rl + cq, 128 * t + mcol] = \
                            conv_w[di, cq - c]
                if t < NT - 1:
                    for rl2 in range(2):
                        di = 4 + rl2 - il
                        if not (0 <= di < KW):
                            continue
                        for cq in range(c, c + KW):
                            spillT[28 * rl2 + cq, 128 * t + mcol] = \
                                conv_w[di, cq - c]
    wsbT = np.zeros((112, NT * 32), np.float32)
    for t in range(NT):
        m = _chunk_m(t)
        wsbT[0:m, 32 * t:32 * t + NCLS] = W[CH * t:CH * t + m, :]
    biascol = np.zeros((128, 1), np.float32)
    for gg in range(4):
        biascol[32 * gg:32 * gg + NCLS, 0] = b
    return mainT, spillT, wsbT, biascol


def _host_pair_constants(conv_w, W):
    """2x2-tiled conv lhsT + matching FC lhsT.

    Bank k: out-row-pair 2k (rows 4k,4k+1) on PSUM partitions 0-63 and
    pair 2k+1 (rows 4k+2,4k+3) on partitions 64-127.  Input rows
    4k,4k+1 live at x partitions 0-55 of block k; rows 4k+2,4k+3 at
    partitions 64-119.  pairW[partition, 128k + 64*half + 26*j + c].
    """
    pairW = np.zeros((128, NT * 128), np.float32)
    wsbP = np.zeros((128, NT * 32), np.float32)
    for k in range(NT):
        for half in range(2):           # 0: pair 2k, 1: pair 2k+1
            p = 2 * k + half
            if p > 12:
                continue
            base_row = 2 * p            # first out row of the pair
            for j in range(2):          # out row base_row + j
                orow = base_row + j
                for c in range(OUT):
                    col = 128 * k + 64 * half + 26 * j + c
                    for di in range(KW):
                        r = orow + di   # input image row
                        rl = r - 4 * k  # row within block k (0..5)
                        for dj in range(KW):
                            cq = c + dj
                            if rl < 2:
                                part = 28 * rl + cq
                            elif rl < 4:
                                part = 64 + 28 * (rl - 2) + cq
                            else:       # rows 4k+4,4k+5 = block k+1 rows 0,1
                                part = 28 * (rl - 4) + cq
                            pairW[part, col] = conv_w[di, dj]
                    feat = OUT * orow + c   # conv feature index
                    wsbP[64 * half + 26 * j + c, 32 * k:32 * k + NCLS] = \
                        W[feat, :]
    return pairW, wsbP


def _marshal_x(x):
    """[B, 784] fp32 -> per-core [NG, 112, 7*512] fp16.

    x_d[core, g, 28*rl + c, 512*t + n] = x[4096*core + 512*g + n,
                                           28*(4t+rl) + c].
    """
    xs = x.reshape(NCORES, NG, N, NT, 4, IMG)    # core g n t rl c
    xs = xs.transpose(0, 1, 4, 5, 3, 2)          # core g rl c t n
    xs = np.ascontiguousarray(xs, dtype=np.float16)
    return xs.reshape(NCORES, NG, 112, NT * N)


def _unmarshal_out(res):
    """Per-core out [2, 128, 512] = (batch, 32*gg + c, n) -> [B, 10].

    sample = 4096*core + 2048*batch + 512*gg + n; rows c>=10 are junk.
    """
    outs = []
    for r in res:
        o = r.reshape(2, 4, 32, N)[:, :, 0:NCLS].transpose(0, 1, 3, 2)
        outs.append(np.ascontiguousarray(o).reshape(B_CORE, NCLS))
    return np.concatenate(outs, axis=0)


def _col_of(t, il, c):
    """Column of output (il, c) in chunk t's permuted layout.

    The spill-affected pair (il 2,3) sits at a 64-aligned slab: cols
    0-51 for even t, 64-115 for odd t; the other pair at the other slab.
    """
    if il >= 2:
        return (0 if t % 2 == 0 else 64) + 26 * (il - 2) + c
    return (64 if t % 2 == 0 else 0) + 26 * il + c


def _host_colspill_constants(conv_w, W):
    """Conv lhsTs with parity-permuted M layout + matching FC lhsT."""
    mainC = np.zeros((112, NT * 128), np.float32)
    spillC = np.zeros((112, (NT - 1) * 64), np.float32)
    wsbF = np.zeros((128, NT * 32), np.float32)
    for t in range(NT):
        nil = 4 if t < NT - 1 else 2
        for il in range(nil):
            for c in range(OUT):
                col = _col_of(t, il, c)
                for rl in range(4):
                    di = rl - il
                    if 0 <= di < KW:
                        for dj in range(KW):
                            mainC[28 * rl + c + dj, 128 * t + col] = \
                                conv_w[di, dj]
                if t < NT - 1 and il >= 2:
                    scol = 64 * t + 26 * (il - 2) + c
                    for rl2 in range(2):
                        di = 4 + rl2 - il
                        if 0 <= di < KW:
                            for dj in range(KW):
                                spillC[28 * rl2 + c + dj, scol] = \
                                    conv_w[di, dj]
                wsbF[col, 32 * t:32 * t + NCLS] = W[OUT * (4 * t + il) + c]
    return mainC, spillC, wsbF


def _make_in_maps(x, conv_w, W, b):
    """Marshaled per-core input maps (superset of any variant's inputs)."""
    mainT, spillT, wsbT, biascol = _host_constants(conv_w, W, b)
    pairW, wsbP = _host_pair_constants(conv_w, W)
    mainC, spillC, wsbF = _host_colspill_constants(conv_w, W)
    xm = _marshal_x(np.ascontiguousarray(np.asarray(x, dtype=np.float32)))
    in_maps = []
    for i in range(NCORES):
        in_maps.append({
            "x": xm[i],
            "mainT": mainT.astype(np.float16),
            "spillT": spillT.astype(np.float16),
            "wsbT": wsbT.astype(np.float16),
            "pairW": pairW.astype(np.float16),
            "wsbP": wsbP.astype(np.float16),
            "mainC": mainC.astype(np.float16),
            "spillC": spillC.astype(np.float16),
            "wsbF": wsbF.astype(np.float16),
            "biascol": biascol,
        })
    return in_maps


def _run(x, conv_w, W, b, trace=False, mm_dtype=F16):
    x = np.ascontiguousarray(np.asarray(x, dtype=np.float32))
    conv_w = np.asarray(conv_w, dtype=np.float32)
    W = np.asarray(W, dtype=np.float32)
    b = np.asarray(b, dtype=np.float32)
    assert x.shape == (B_TOTAL, NPIX), x.shape

    key = ("prog", str(mm_dtype))
    if key not in _CACHE:
        _CACHE[key] = _build_program(mm_dtype)
    nc = _CACHE[key]

    in_maps = _make_in_maps(x, conv_w, W, b)
    res = run_bass_kernel_spmd(nc, in_maps, core_ids=list(range(NCORES)),
                               trace=trace)
    out = _unmarshal_out([res.results[i]["out"] for i in range(NCORES)])
    return out, res


def kernel(x, conv_w, W, b):
    out, _ = _run(x, conv_w, W, b, trace=False)
    return out
